# revision 2
# baseline (speedup 1.0000x reference)
"""Multi-head attention with QK-LayerNorm on 8 Trainium2 NeuronCores.

Problem: B=2, S=F=2048, D=1024, H=16, HD=64 (fp32).
    q = LN_head(x_q @ Wq) * HD^-0.5 ; k = LN_head(x_k @ Wk) ; v = x_v @ Wv
    ctx = softmax(q k^T) v ; out = LN(ctx) @ Wproj

Sharding (tensor-parallel over heads): core c owns heads [2c, 2c+1]
(d-columns [128c:128c+128]) for BOTH batches.  Each core receives the
full transposed inputs x^T (both batches, bf16) plus its 128-column
weight slices, computes q/k/v projections directly in transposed
layout, runs attention for its 2 heads over all 4096 query rows, and
normalizes ctx by the softmax denominator.  A single 8-core AllToAll
(1 MB, bf16) re-shards ctx from head-split to row-split; core c ends
with ctx rows [batch c//4, 512*(c%4):...] x full D, then does the
output LayerNorm + projection locally.  No AllGather of k/v is needed
because every core computes its own heads' k/v from the full input.

QK-LN mean is folded into the weights on the host (per-head column
centering makes the projection mean-free), so only the variance is
computed on device (PE block-ones matmul + DVE Newton rsqrt; the ACT
engine runs exp exclusively - one table set, no switches).  Out-LN
gamma/beta are folded into Wproj / a bias vector on the host.
"""

import numpy as np
import ml_dtypes

import concourse.bass as bass
import concourse.mybir as mybir
import concourse.tile as tile
from concourse import bacc, bass_utils

F32 = mybir.dt.float32
F32R = mybir.dt.float32r
BF16 = mybir.dt.bfloat16
I32 = mybir.dt.int32
AF = mybir.ActivationFunctionType
OP = mybir.AluOpType

B, S, F, D, H, HD = 2, 2048, 2048, 1024, 16, 64
EPS = 1e-5
NCORES = 8
HPC = 2                    # heads per core
DPC = HPC * HD             # 128 d-columns per core
KC = D // 128              # 8 contraction chunks
SBLK = 1024                # projection s-block
NSB = S // SBLK            # 2
NBLK = B * NSB             # 4 projection blocks per tensor
ABLK = 512                 # attention s-block
NAB = S // ABLK            # 4
FCN = F // 128             # 16 f-chunks
OUT_ROWS = B * S // NCORES  # 512 output rows per core
MAGIC = 0x5F3759DF


def _rsqrt_dve(nc, pool, u, nparts, ncols, tag):
    """y = 1/sqrt(u) via bit trick + 2 Newton steps, all on DVE.

    u: SBUF F32 AP (nparts, ncols), strictly positive. Returns F32R tile.
    """
    yt = pool.tile([nparts, ncols], F32, tag=f"{tag}_y")
    nc.vector.tensor_scalar(
        out=yt.bitcast(I32), in0=u.bitcast(I32),
        scalar1=1, scalar2=None, op0=OP.logical_shift_right)
    nc.vector.tensor_scalar(
        out=yt.bitcast(I32), in0=yt.bitcast(I32),
        scalar1=-1, scalar2=MAGIC, op0=OP.mult, op1=OP.add)
    yr = pool.tile([nparts, ncols], F32R, tag=f"{tag}_yr")
    for it in range(2):
        t2 = pool.tile([nparts, ncols], F32, tag=f"{tag}_t")
        nc.vector.tensor_mul(t2, yt, yt)
        nc.vector.tensor_mul(t2, t2, u)
        nc.vector.tensor_scalar(
            out=t2, in0=t2, scalar1=-0.5, scalar2=1.5, op0=OP.mult, op1=OP.add)
        if it == 0:
            nc.vector.tensor_mul(yt, yt, t2)
        else:
            with nc.allow_low_precision(reason="fp32r is 32-bit storage"):
                nc.vector.tensor_mul(yr, yt, t2)
    return yr


def build(n_repeat=1):
    nc = bacc.Bacc(None, target_bir_lowering=False)

    xqT = nc.declare_dram_parameter("xqT", [B, D, S], BF16, isOutput=False)
    xkT = nc.declare_dram_parameter("xkT", [B, D, S], BF16, isOutput=False)
    xvT = nc.declare_dram_parameter("xvT", [B, D, S], BF16, isOutput=False)
    wq = nc.declare_dram_parameter("wq", [D, DPC], BF16, isOutput=False)
    wk = nc.declare_dram_parameter("wk", [D, DPC], BF16, isOutput=False)
    wv = nc.declare_dram_parameter("wv", [D, DPC], BF16, isOutput=False)
    wp = nc.declare_dram_parameter("wp", [D, D], F32, isOutput=False)
    bv = nc.declare_dram_parameter("bv", [D], F32, isOutput=False)
    qg = nc.declare_dram_parameter("qg", [DPC], F32, isOutput=False)  # *scale
    qb = nc.declare_dram_parameter("qb", [DPC], F32, isOutput=False)  # *scale
    kg = nc.declare_dram_parameter("kg", [DPC], F32, isOutput=False)
    kb = nc.declare_dram_parameter("kb", [DPC], F32, isOutput=False)
    bo2 = nc.declare_dram_parameter("bo2", [128, 2], F32, isOutput=False)
    idp = nc.declare_dram_parameter("idp", [128, 128], BF16, isOutput=False)
    cb2 = nc.declare_dram_parameter("cb2", [2, 128], F32, isOutput=False)
    out = nc.declare_dram_parameter("out", [OUT_ROWS, D], F32, isOutput=True)

    with tile.TileContext(nc) as tc:
        with (
            tc.tile_pool(name="const", bufs=1) as const,
            tc.tile_pool(name="persist", bufs=1) as persist,
            tc.tile_pool(name="dram", bufs=1, space="DRAM") as dram,
        ):
            # ---- constants ----
            ident_bf = const.tile([128, 128], BF16)
            nc.gpsimd.dma_start(out=ident_bf, in_=idp[:, :])
            ones_row_f = const.tile([1, 128], F32)
            nc.vector.memset(ones_row_f, 1.0)
            ones_row = const.tile([1, 128], F32R)
            nc.vector.tensor_copy(ones_row, ones_row_f)
            ones_col_bf = const.tile([128, 1], BF16)
            nc.vector.memset(ones_col_bf, 1.0)
            ones_col_f = const.tile([128, 1], F32)
            nc.vector.memset(ones_col_f, 1.0)
            ones_col_r = const.tile([128, 1], F32R)
            nc.vector.tensor_copy(ones_col_r, ones_col_f)
            # block-ones (128, 2): col h = 1 on partitions [64h, 64h+64)
            blockones = const.tile([128, 2], F32R)
            nc.gpsimd.dma_start(out=blockones, in_=bo2[:, :].bitcast(F32R))
            # bcast (2, 128): row h = 1 on cols [64h, 64h+64)
            bc2 = const.tile([2, 128], F32R)
            nc.gpsimd.dma_start(out=bc2, in_=cb2[:, :].bitcast(F32R))

            def gb_tile(param):
                t = const.tile([128, 1], F32, tag=f"gb_{param.name}")
                nc.gpsimd.dma_start(
                    out=t, in_=param.rearrange("(p one) -> p one", one=1))
                return t

            qg_t, qb_t = gb_tile(qg), gb_tile(qb)
            kg_t, kb_t = gb_tile(kg), gb_tile(kb)
            bv_t = const.tile([1, D], F32R)
            nc.gpsimd.dma_start(
                out=bv_t,
                in_=bv.rearrange("(one n) -> one n", one=1).bitcast(F32R))

            for _rep in range(n_repeat):
                # ---- persistent SBUF ----
                qT = persist.tile([128, B, S], BF16, tag="qT")
                kT = persist.tile([128, B, S], BF16, tag="kT")
                vx = persist.tile([128, B, FCN, HPC, HD + 1], BF16, tag="vx")
                ctxF = persist.tile([128, KC, OUT_ROWS], BF16, tag="ctxF")
                nc.vector.memset(vx[:, :, :, :, 0:1], 1.0)

                def load_w(param, tag):
                    t = persist.tile([128, KC, DPC], BF16, tag=tag)
                    nc.sync.dma_start(
                        out=t, in_=param.rearrange("(kc p) n -> p kc n", p=128))
                    return t

                wk_t = load_w(wk, "wk")
                wq_t = load_w(wq, "wq")
                wv_t = load_w(wv, "wv")
                wp_t = persist.tile([128, KC, D], F32R, tag="wp")

                # ---- DRAM all-to-all buffers ----
                a2a_in = dram.tile([NCORES, 128, OUT_ROWS], BF16, tag="a2ai",
                                   name="a2ai")
                a2a_out = dram.tile([NCORES, 128, OUT_ROWS], BF16, tag="a2ao",
                                    name="a2ao")

                # ================= phase 1: projections =================
                with (
                    tc.tile_pool(name="xs", bufs=10) as xs,
                    tc.tile_pool(name="qcp", bufs=8) as qcp,
                    tc.tile_pool(name="p1sb", bufs=2) as p1sb,
                    tc.tile_pool(name="p1st", bufs=1) as p1st,
                    tc.tile_pool(name="p1u", bufs=2) as p1u,
                    tc.tile_pool(name="p1ps", bufs=3, space="PSUM") as p1ps,
                    tc.tile_pool(name="tpps", bufs=2, space="PSUM") as tpps,
                ):
                    def qk_sweep1(xparam, w_t):
                        """Dense proj matmuls + variance, one-block pipeline."""
                        u_all = p1u.tile([2, NBLK, SBLK], F32, tag="u")
                        qcs, pss = [], []

                        def mm_block(i):
                            b, sb = divmod(i, NSB)
                            c0 = sb * SBLK
                            ps = p1ps.tile([128, SBLK], F32, tag="ps",
                                           name="ps")
                            for kc in range(KC):
                                xt = xs.tile([128, SBLK], BF16, tag="x")
                                nc.sync.dma_start(
                                    out=xt,
                                    in_=xparam[b, kc * 128:(kc + 1) * 128,
                                               c0:c0 + SBLK])
                                for nh in range(2):
                                    nc.tensor.matmul(
                                        ps[:, nh * 512:(nh + 1) * 512],
                                        w_t[:, kc, :],
                                        xt[:, nh * 512:(nh + 1) * 512],
                                        start=(kc == 0), stop=(kc == KC - 1))
                            pss.append(ps)

                        def var_block(i):
                            ps = pss[i]
                            qc = qcp.tile([128, SBLK], BF16, tag="qc",
                                          name="qc")
                            nc.scalar.copy(qc, ps[:, :])
                            qcs.append(qc)
                            sq = p1sb.tile([128, SBLK], F32R, tag="sq")
                            nc.vector.tensor_mul(sq, qc, qc)
                            var_ps = p1ps.tile([128, SBLK], F32, tag="ps",
                                               name="var_ps")
                            for nh in range(2):
                                nc.tensor.matmul(
                                    var_ps[0:2, nh * 512:(nh + 1) * 512],
                                    blockones,
                                    sq[:, nh * 512:(nh + 1) * 512],
                                    start=True, stop=True)
                            nc.scalar.activation(
                                u_all[:, i, :], var_ps[0:2, :], AF.Copy,
                                bias=EPS, scale=1.0 / HD)

                        mm_block(0)
                        for i in range(1, NBLK):
                            mm_block(i)
                            var_block(i - 1)
                        var_block(NBLK - 1)
                        return u_all, qcs

                    def qk_sweep23(u_all, qcs, g_t, b_t, dstT):
                        """Full-lane rsqrt, then broadcast + apply."""
                        nlane = NBLK * SBLK * 2 // 128
                        u128 = p1st.tile([128, nlane], F32, tag="u128")
                        nc.gpsimd.dma_start(
                            out=u128[:, :],
                            in_=u_all.rearrange("p i n -> p (i n)"))
                        r128 = _rsqrt_dve(nc, p1st, u128, 128, nlane, "ln")
                        rstd2 = p1st.tile([2, NBLK, SBLK], F32R, tag="rstd2")
                        nc.gpsimd.dma_start(
                            out=rstd2.rearrange("p i n -> p (i n)"),
                            in_=r128[:, :])
                        for i in range(NBLK):
                            b, sb = divmod(i, NSB)
                            c0 = sb * SBLK
                            rb_ps = p1ps.tile([128, SBLK], F32, tag="ps",
                                              name="rb_ps")
                            for nh in range(2):
                                nc.tensor.matmul(
                                    rb_ps[:, nh * 512:(nh + 1) * 512], bc2,
                                    rstd2[:, i, nh * 512:(nh + 1) * 512],
                                    start=True, stop=True)
                            t1 = p1sb.tile([128, SBLK], F32, tag="t1")
                            nc.vector.scalar_tensor_tensor(
                                out=t1, in0=qcs[i], scalar=g_t,
                                in1=rb_ps[:, :],
                                op0=OP.mult, op1=OP.mult)
                            nc.vector.tensor_scalar(
                                out=dstT[:, b, c0:c0 + SBLK], in0=t1,
                                scalar1=b_t, scalar2=None, op0=OP.add)

                    def proj_v():
                        for b in range(B):
                            for sb in range(NSB):
                                c0 = sb * SBLK
                                ps = p1ps.tile([128, SBLK], F32, tag="ps",
                                               name="ps")
                                for kc in range(KC):
                                    xt = xs.tile([128, SBLK], BF16, tag="x")
                                    nc.sync.dma_start(
                                        out=xt,
                                        in_=xvT[b, kc * 128:(kc + 1) * 128,
                                                c0:c0 + SBLK])
                                    for nh in range(2):
                                        nc.tensor.matmul(
                                            ps[:, nh * 512:(nh + 1) * 512],
                                            wv_t[:, kc, :],
                                            xt[:, nh * 512:(nh + 1) * 512],
                                            start=(kc == 0),
                                            stop=(kc == KC - 1))
                                vts = p1sb.tile([128, SBLK], BF16, tag="vts")
                                nc.scalar.copy(vts, ps[:, :])
                                for q4 in range(2):
                                    tp = tpps.tile([128, 512], BF16, tag="tp")
                                    for j in range(4):
                                        nc.tensor.transpose(
                                            tp[:, j * 128:(j + 1) * 128],
                                            vts[:, (q4 * 4 + j) * 128:
                                                (q4 * 4 + j + 1) * 128],
                                            ident_bf[:, :])
                                    fc0 = sb * 8 + q4 * 4
                                    nc.vector.tensor_copy(
                                        vx[:, b, fc0:fc0 + 4, :, 1:HD + 1],
                                        tp[:, :].rearrange(
                                            "p (j h d) -> p j h d",
                                            h=HPC, d=HD))

                    uk, qck = qk_sweep1(xkT, wk_t)
                    uq, qcq = qk_sweep1(xqT, wq_t)
                    qk_sweep23(uk, qck, kg_t, kb_t, kT)
                    proj_v()
                    qk_sweep23(uq, qcq, qg_t, qb_t, qT)

                # ================= phase 2: attention =================
                with (
                    tc.tile_pool(name="att", bufs=3) as att,
                    tc.tile_pool(name="attps", bufs=2, space="PSUM") as attps,
                    tc.tile_pool(name="ctxps", bufs=2, space="PSUM") as ctxps,
                ):
                    def normalize(blk, ctx_ps):
                        b, ab = divmod(blk, NAB)
                        for hh in range(2):
                            den = att.tile([1, ABLK], F32R, tag="den")
                            with nc.allow_low_precision(
                                    reason="fp32r is 32-bit storage"):
                                nc.vector.reciprocal(den, ctx_ps[hh][0:1, :])
                            bc_ps = attps.tile([128, 2 * ABLK], F32,
                                               tag="sp", name="bc_ps")
                            nc.tensor.matmul(
                                bc_ps[0:HD + 1, 0:ABLK],
                                ones_row[:, 0:HD + 1], den,
                                start=True, stop=True)
                            rbc = att.tile([HD + 1, ABLK], F32, tag="rbc")
                            nc.vector.tensor_copy(rbc, bc_ps[0:HD + 1, 0:ABLK])
                            tmp = att.tile([HD + 1, ABLK], BF16, tag="ctmp")
                            nc.vector.tensor_mul(tmp, ctx_ps[hh][:, :], rbc)
                            nc.gpsimd.dma_start(
                                out=a2a_in[blk][hh * 64:(hh + 1) * 64, :],
                                in_=tmp[1:HD + 1, :])

                    prev = None
                    for blk in range(B * NAB):
                        b, ab = divmod(blk, NAB)
                        s0 = ab * ABLK
                        ctx_ps = [
                            ctxps.tile([HD + 1, ABLK], F32, tag="ctxA",
                                       name="ctxA"),
                            ctxps.tile([HD + 1, ABLK], F32, tag="ctxB",
                                       name="ctxB"),
                        ]
                        for sc in range(FCN // 2):
                            sp = [attps.tile([128, 2 * ABLK], F32,
                                             tag="sp", name="sp")
                                  for _ in range(2)]
                            for cc in range(2):
                                fc = sc * 2 + cc
                                for hh in range(2):
                                    nc.tensor.matmul(
                                        sp[hh][:, cc * ABLK:(cc + 1) * ABLK],
                                        kT[hh * 64:(hh + 1) * 64, b,
                                           fc * 128:(fc + 1) * 128],
                                        qT[hh * 64:(hh + 1) * 64, b,
                                           s0:s0 + ABLK],
                                        start=True, stop=True,
                                        tile_position=(hh * 64, 0))
                            pt = []
                            for hh in range(2):
                                p = att.tile([128, 2 * ABLK], BF16,
                                             tag="pt", name="pt")
                                nc.scalar.activation(p, sp[hh][:, :], AF.Exp)
                                pt.append(p)
                            for cc in range(2):
                                fc = sc * 2 + cc
                                for hh in range(2):
                                    nc.tensor.matmul(
                                        ctx_ps[hh][:, :],
                                        vx[:, b, fc, hh, :],
                                        pt[hh][:, cc * ABLK:(cc + 1) * ABLK],
                                        start=(fc == 0),
                                        stop=(fc == FCN - 1))
                            if sc == 0 and prev is not None:
                                normalize(blk - 1, prev)
                        prev = ctx_ps
                    normalize(B * NAB - 1, prev)

                # Wproj load: deferred so its 4 MB DMA rides P2's idle
                # DMA window instead of competing with the x-streams
                nc.gpsimd.dma_start(
                    out=wp_t,
                    in_=wp.rearrange("(kc p) n -> p kc n", p=128).bitcast(F32R))

                # ================= all-to-all re-shard =================
                nc.gpsimd.collective_compute(
                    "AllToAll", mybir.AluOpType.bypass,
                    replica_groups=[list(range(NCORES))],
                    ins=[a2a_in.opt()], outs=[a2a_out.opt()],
                )
                nc.sync.dma_start(
                    out=ctxF, in_=a2a_out.rearrange("j p r -> p j r"))

                # ============ phase 3: out-LN + projection ============
                with (
                    tc.tile_pool(name="p3", bufs=2) as p3,
                    tc.tile_pool(name="p3w", bufs=1) as p3w,
                    tc.tile_pool(name="p3s", bufs=1) as p3s,
                    tc.tile_pool(name="st_ps", bufs=1, space="PSUM") as st_ps,
                    tc.tile_pool(name="o_ps", bufs=2, space="PSUM") as o_ps,
                ):
                    # bias vector broadcast (beta @ Wproj), computed once
                    bvb_ps = o_ps.tile([128, D], F32, tag="o", name="bvb_ps")
                    for nh in range(2):
                        nc.tensor.matmul(
                            bvb_ps[:, nh * 512:(nh + 1) * 512], ones_row,
                            bv_t[:, nh * 512:(nh + 1) * 512],
                            start=True, stop=True)
                    bvb = p3w.tile([128, D], F32, tag="bvb_sb")
                    nc.vector.tensor_copy(bvb, bvb_ps[:, :])

                    sum_ps = st_ps.tile([1, OUT_ROWS], F32, tag="sum")
                    for kc in range(KC):
                        nc.tensor.matmul(
                            sum_ps[:, :], ones_col_bf, ctxF[:, kc, :],
                            start=(kc == 0), stop=(kc == KC - 1))
                    sq_ps = st_ps.tile([1, OUT_ROWS], F32, tag="sq")
                    for kc in range(KC):
                        sq = p3.tile([128, OUT_ROWS], F32R, tag="sq")
                        nc.vector.tensor_mul(sq, ctxF[:, kc, :],
                                             ctxF[:, kc, :])
                        nc.tensor.matmul(
                            sq_ps[:, :], ones_col_r, sq,
                            start=(kc == 0), stop=(kc == KC - 1))
                    # copy sums to SBUF, reshape to 128 lanes for the
                    # small-stat math (DVE lanes are per-partition);
                    # DMAs pair elements in linearized AP order, so a
                    # (1,512) <-> (128,4) move is just a reshape.
                    ssb = p3s.tile([1, 2, OUT_ROWS], F32, tag="ssb")
                    nc.scalar.copy(ssb[:, 0, :], sum_ps[:, :])
                    nc.scalar.copy(ssb[:, 1, :], sq_ps[:, :])
                    s_sum = p3s.tile([128, 4], F32, tag="s_sum")
                    nc.gpsimd.dma_start(out=s_sum[:, :], in_=ssb[:, 0, :])
                    s_sq = p3s.tile([128, 4], F32, tag="s_sq")
                    nc.gpsimd.dma_start(out=s_sq[:, :], in_=ssb[:, 1, :])
                    mean8 = p3s.tile([128, 4], F32, tag="mean8")
                    nc.vector.tensor_scalar_mul(mean8, s_sum[:, :], 1.0 / D)
                    m28 = p3s.tile([128, 4], F32, tag="m28")
                    nc.vector.tensor_mul(m28, mean8, mean8)
                    u8 = p3s.tile([128, 4], F32, tag="u8")
                    nc.vector.tensor_scalar(
                        out=u8, in0=s_sq[:, :], scalar1=1.0 / D, scalar2=EPS,
                        op0=OP.mult, op1=OP.add)
                    nc.vector.tensor_sub(u8, u8, m28)
                    r8 = _rsqrt_dve(nc, p3s, u8, 128, 4, "oln")
                    negm8 = p3s.tile([128, 4], F32R, tag="negm8")
                    with nc.allow_low_precision(
                            reason="fp32r is 32-bit storage"):
                        nc.vector.tensor_mul(negm8, mean8, r8.bitcast(F32))
                        nc.vector.tensor_scalar_mul(negm8, negm8, -1.0)
                    rstd = p3s.tile([1, OUT_ROWS], F32R, tag="rstd")
                    nc.gpsimd.dma_start(out=rstd[:, :], in_=r8[:, :])
                    negm = p3s.tile([1, OUT_ROWS], F32R, tag="negm")
                    nc.gpsimd.dma_start(out=negm[:, :], in_=negm8[:, :])
                    rstd_ps = st_ps.tile([128, OUT_ROWS], F32, tag="rstd_ps")
                    nc.tensor.matmul(
                        rstd_ps[:, :], ones_row, rstd,
                        start=True, stop=True)
                    negm_ps = st_ps.tile([128, OUT_ROWS], F32, tag="negm_ps")
                    nc.tensor.matmul(
                        negm_ps[:, :], ones_row, negm,
                        start=True, stop=True)

                    ctxn = p3w.tile([128, KC, OUT_ROWS], F32R, tag="ctxn")
                    for kc in range(KC):
                        t = p3.tile([128, OUT_ROWS], F32, tag="lnt")
                        nc.vector.tensor_mul(t, ctxF[:, kc, :], rstd_ps[:, :])
                        with nc.allow_low_precision(
                                reason="fp32r is 32-bit storage"):
                            nc.vector.tensor_add(ctxn[:, kc, :], t,
                                                 negm_ps[:, :])

                    for m in range(OUT_ROWS // 128):
                        ps = o_ps.tile([128, D], F32, tag="o")
                        for n in range(2):
                            for kc in range(KC):
                                nc.tensor.matmul(
                                    ps[:, n * 512:(n + 1) * 512],
                                    ctxn[:, kc, m * 128:(m + 1) * 128],
                                    wp_t[:, kc, n * 512:(n + 1) * 512],
                                    start=(kc == 0), stop=(kc == KC - 1))
                        o_sb = p3.tile([128, D], F32, tag="osb")
                        nc.vector.tensor_add(o_sb, ps[:, :], bvb)
                        nc.sync.dma_start(
                            out=out[m * 128:(m + 1) * 128, :], in_=o_sb)
    nc.finalize()
    return nc


def _center_per_head(W):
    Wc = np.asarray(W, np.float32).reshape(D, H, HD)
    return (Wc - Wc.mean(axis=2, keepdims=True)).reshape(D, D)


def make_in_maps(x_q, x_k, x_v, Wq, Wk, Wv, Wproj,
                 q_gamma, q_beta, k_gamma, k_beta, out_gamma, out_beta):
    """Host-side prep: transpose inputs, center + slice weights, fold scale,
    fold out-LN gamma/beta into Wproj / a bias vector."""
    bf = ml_dtypes.bfloat16
    scale = np.float32(HD ** -0.5)
    xqT = np.ascontiguousarray(
        np.asarray(x_q, np.float32).transpose(0, 2, 1)).astype(bf)
    xkT = np.ascontiguousarray(
        np.asarray(x_k, np.float32).transpose(0, 2, 1)).astype(bf)
    xvT = np.ascontiguousarray(
        np.asarray(x_v, np.float32).transpose(0, 2, 1)).astype(bf)
    WqC = _center_per_head(Wq)
    WkC = _center_per_head(Wk)
    Wv = np.ascontiguousarray(np.asarray(Wv, np.float32))
    Wp = np.asarray(Wproj, np.float32)
    og = np.asarray(out_gamma, np.float32)
    ob = np.asarray(out_beta, np.float32)
    Wp2 = np.ascontiguousarray(Wp * og[:, None])
    bvec = ob @ Wp
    qg2 = np.tile(np.asarray(q_gamma, np.float32) * scale, HPC)
    qb2 = np.tile(np.asarray(q_beta, np.float32) * scale, HPC)
    kg2 = np.tile(np.asarray(k_gamma, np.float32), HPC)
    kb2 = np.tile(np.asarray(k_beta, np.float32), HPC)
    bo2 = np.zeros((128, 2), np.float32)
    bo2[0:64, 0] = 1.0
    bo2[64:128, 1] = 1.0
    cb2 = np.zeros((2, 128), np.float32)
    cb2[0, 0:64] = 1.0
    cb2[1, 64:128] = 1.0
    idp = np.eye(128, dtype=np.float32).astype(bf)
    in_maps = []
    for c in range(NCORES):
        dc = slice(DPC * c, DPC * (c + 1))
        in_maps.append({
            "xqT": xqT, "xkT": xkT, "xvT": xvT,
            "wq": np.ascontiguousarray(WqC[:, dc]).astype(bf),
            "wk": np.ascontiguousarray(WkC[:, dc]).astype(bf),
            "wv": np.ascontiguousarray(Wv[:, dc]).astype(bf),
            "wp": Wp2, "bv": bvec,
            "qg": qg2, "qb": qb2, "kg": kg2, "kb": kb2,
            "bo2": bo2, "cb2": cb2, "idp": idp,
        })
    return in_maps


_NC_CACHE = None


def _get_nc():
    global _NC_CACHE
    if _NC_CACHE is None:
        _NC_CACHE = build(1)
    return _NC_CACHE


def kernel(x_q, x_k, x_v, Wq, Wk, Wv, Wproj,
           q_gamma, q_beta, k_gamma, k_beta, out_gamma, out_beta,
           _trace=False):
    in_maps = make_in_maps(x_q, x_k, x_v, Wq, Wk, Wv, Wproj,
                           q_gamma, q_beta, k_gamma, k_beta,
                           out_gamma, out_beta)
    nc = _get_nc()
    res = bass_utils.run_bass_kernel_spmd(
        nc, in_maps, list(range(NCORES)), trace=_trace)
    full = np.empty((B, S, D), dtype=np.float32)
    for c in range(NCORES):
        b, r = c // 4, c % 4
        full[b, r * OUT_ROWS:(r + 1) * OUT_ROWS, :] = res.results[c]["out"]
    if _trace:
        return full, res
    return full


# revision 3
# speedup vs baseline: 1.0460x; 1.0460x over previous
"""Multi-head attention with QK-LayerNorm on 8 Trainium2 NeuronCores.

Problem: B=2, S=F=2048, D=1024, H=16, HD=64 (fp32).
    q = LN_head(x_q @ Wq) * HD^-0.5 ; k = LN_head(x_k @ Wk) ; v = x_v @ Wv
    ctx = softmax(q k^T) v ; out = LN(ctx) @ Wproj

Sharding (tensor-parallel over heads): core c owns heads [2c, 2c+1]
(d-columns [128c:128c+128]) for BOTH batches.  Each core receives the
full transposed inputs x^T (both batches, bf16) plus its 128-column
weight slices, computes q/k/v projections directly in transposed
layout, runs attention for its 2 heads over all 4096 query rows, and
normalizes ctx by the softmax denominator.  A single 8-core AllToAll
(1 MB, bf16) re-shards ctx from head-split to row-split; core c ends
with ctx rows [batch c//4, 512*(c%4):...] x full D, then does the
output LayerNorm + projection locally.  No AllGather of k/v is needed
because every core computes its own heads' k/v from the full input.

QK-LN mean is folded into the weights on the host (per-head column
centering makes the projection mean-free), so only the variance is
computed on device (PE block-ones matmul + DVE Newton rsqrt; the ACT
engine runs exp exclusively - one table set, no switches).  Out-LN
gamma/beta are folded into Wproj / a bias vector on the host.
"""

import numpy as np
import ml_dtypes

import concourse.bass as bass
import concourse.mybir as mybir
import concourse.tile as tile
from concourse import bacc, bass_utils

F32 = mybir.dt.float32
F32R = mybir.dt.float32r
BF16 = mybir.dt.bfloat16
I32 = mybir.dt.int32
AF = mybir.ActivationFunctionType
OP = mybir.AluOpType

B, S, F, D, H, HD = 2, 2048, 2048, 1024, 16, 64
EPS = 1e-5
NCORES = 8
HPC = 2                    # heads per core
DPC = HPC * HD             # 128 d-columns per core
KC = D // 128              # 8 contraction chunks
SBLK = 1024                # projection s-block
NSB = S // SBLK            # 2
NBLK = B * NSB             # 4 projection blocks per tensor
ABLK = 512                 # attention s-block
NAB = S // ABLK            # 4
FCN = F // 128             # 16 f-chunks
OUT_ROWS = B * S // NCORES  # 512 output rows per core
MAGIC = 0x5F3759DF


def _rsqrt_dve(nc, pool, u, nparts, ncols, tag):
    """y = 1/sqrt(u) via bit trick + 2 Newton steps, all on DVE.

    u: SBUF F32 AP (nparts, ncols), strictly positive. Returns F32R tile.
    """
    yt = pool.tile([nparts, ncols], F32, tag=f"{tag}_y")
    nc.vector.tensor_scalar(
        out=yt.bitcast(I32), in0=u.bitcast(I32),
        scalar1=1, scalar2=None, op0=OP.logical_shift_right)
    nc.vector.tensor_scalar(
        out=yt.bitcast(I32), in0=yt.bitcast(I32),
        scalar1=-1, scalar2=MAGIC, op0=OP.mult, op1=OP.add)
    yr = pool.tile([nparts, ncols], F32R, tag=f"{tag}_yr")
    for it in range(2):
        t2 = pool.tile([nparts, ncols], F32, tag=f"{tag}_t")
        nc.vector.tensor_mul(t2, yt, yt)
        nc.vector.tensor_mul(t2, t2, u)
        nc.vector.tensor_scalar(
            out=t2, in0=t2, scalar1=-0.5, scalar2=1.5, op0=OP.mult, op1=OP.add)
        if it == 0:
            nc.vector.tensor_mul(yt, yt, t2)
        else:
            with nc.allow_low_precision(reason="fp32r is 32-bit storage"):
                nc.vector.tensor_mul(yr, yt, t2)
    return yr


def build(n_repeat=1):
    nc = bacc.Bacc(None, target_bir_lowering=False)

    xqT = nc.declare_dram_parameter("xqT", [B, D, S], BF16, isOutput=False)
    xkT = nc.declare_dram_parameter("xkT", [B, D, S], BF16, isOutput=False)
    xvT = nc.declare_dram_parameter("xvT", [B, D, S], BF16, isOutput=False)
    wq = nc.declare_dram_parameter("wq", [D, DPC], BF16, isOutput=False)
    wk = nc.declare_dram_parameter("wk", [D, DPC], BF16, isOutput=False)
    wv = nc.declare_dram_parameter("wv", [D, DPC], BF16, isOutput=False)
    wp = nc.declare_dram_parameter("wp", [D, D], F32, isOutput=False)
    bv = nc.declare_dram_parameter("bv", [D], F32, isOutput=False)
    qg = nc.declare_dram_parameter("qg", [DPC], F32, isOutput=False)  # *scale
    qb = nc.declare_dram_parameter("qb", [DPC], F32, isOutput=False)  # *scale
    kg = nc.declare_dram_parameter("kg", [DPC], F32, isOutput=False)
    kb = nc.declare_dram_parameter("kb", [DPC], F32, isOutput=False)
    bo2 = nc.declare_dram_parameter("bo2", [128, 2], F32, isOutput=False)
    idp = nc.declare_dram_parameter("idp", [128, 128], BF16, isOutput=False)
    cb2 = nc.declare_dram_parameter("cb2", [2, 128], F32, isOutput=False)
    out = nc.declare_dram_parameter("out", [OUT_ROWS, D], F32, isOutput=True)

    with tile.TileContext(nc) as tc:
        with (
            tc.tile_pool(name="const", bufs=1) as const,
            tc.tile_pool(name="persist", bufs=1) as persist,
            tc.tile_pool(name="dram", bufs=1, space="DRAM") as dram,
        ):
            # ---- constants ----
            ident_bf = const.tile([128, 128], BF16)
            nc.gpsimd.dma_start(out=ident_bf, in_=idp[:, :])
            ones_row_f = const.tile([1, 128], F32)
            nc.vector.memset(ones_row_f, 1.0)
            ones_row = const.tile([1, 128], F32R)
            nc.vector.tensor_copy(ones_row, ones_row_f)
            ones_col_bf = const.tile([128, 1], BF16)
            nc.vector.memset(ones_col_bf, 1.0)
            ones_col_f = const.tile([128, 1], F32)
            nc.vector.memset(ones_col_f, 1.0)
            ones_col_r = const.tile([128, 1], F32R)
            nc.vector.tensor_copy(ones_col_r, ones_col_f)
            # block-ones (128, 2): col h = 1 on partitions [64h, 64h+64)
            blockones = const.tile([128, 2], F32R)
            nc.gpsimd.dma_start(out=blockones, in_=bo2[:, :].bitcast(F32R))
            # bcast (2, 128): row h = 1 on cols [64h, 64h+64)
            bc2 = const.tile([2, 128], F32R)
            nc.gpsimd.dma_start(out=bc2, in_=cb2[:, :].bitcast(F32R))

            def gb_tile(param):
                t = const.tile([128, 1], F32, tag=f"gb_{param.name}")
                nc.gpsimd.dma_start(
                    out=t, in_=param.rearrange("(p one) -> p one", one=1))
                return t

            qg_t, qb_t = gb_tile(qg), gb_tile(qb)
            kg_t, kb_t = gb_tile(kg), gb_tile(kb)
            bv_t = const.tile([1, D], F32R)
            nc.gpsimd.dma_start(
                out=bv_t,
                in_=bv.rearrange("(one n) -> one n", one=1).bitcast(F32R))

            for _rep in range(n_repeat):
                # ---- persistent SBUF ----
                qT = persist.tile([128, B, S], BF16, tag="qT")
                kT = persist.tile([128, B, S], BF16, tag="kT")
                vx = persist.tile([128, B, FCN, HPC, HD + 1], BF16, tag="vx")
                ctxF = persist.tile([128, KC, OUT_ROWS], BF16, tag="ctxF")
                nc.vector.memset(vx[:, :, :, :, 0:1], 1.0)

                def load_w(param, tag):
                    t = persist.tile([128, KC, DPC], BF16, tag=tag)
                    nc.sync.dma_start(
                        out=t, in_=param.rearrange("(kc p) n -> p kc n", p=128))
                    return t

                wk_t = load_w(wk, "wk")
                wq_t = load_w(wq, "wq")
                wv_t = load_w(wv, "wv")
                wp_t = persist.tile([128, KC, D], F32R, tag="wp")

                # ---- DRAM all-to-all buffers ----
                a2a_in = dram.tile([NCORES, 128, OUT_ROWS], BF16, tag="a2ai",
                                   name="a2ai")
                a2a_out = dram.tile([NCORES, 128, OUT_ROWS], BF16, tag="a2ao",
                                    name="a2ao")

                # ================= phase 1: projections =================
                with (
                    tc.tile_pool(name="xs", bufs=10) as xs,
                    tc.tile_pool(name="qcp", bufs=8) as qcp,
                    tc.tile_pool(name="p1sb", bufs=2) as p1sb,
                    tc.tile_pool(name="p1st", bufs=1) as p1st,
                    tc.tile_pool(name="p1u", bufs=2) as p1u,
                    tc.tile_pool(name="p1ps", bufs=3, space="PSUM") as p1ps,
                    tc.tile_pool(name="tpps", bufs=2, space="PSUM") as tpps,
                ):
                    def qk_sweep1(xparam, w_t):
                        """Dense proj matmuls + variance, one-block pipeline."""
                        u_all = p1u.tile([2, NBLK, SBLK], F32, tag="u")
                        qcs, pss = [], []

                        def mm_block(i):
                            b, sb = divmod(i, NSB)
                            c0 = sb * SBLK
                            ps = p1ps.tile([128, SBLK], F32, tag="ps",
                                           name="ps")
                            for kc in range(KC):
                                xt = xs.tile([128, SBLK], BF16, tag="x")
                                nc.sync.dma_start(
                                    out=xt,
                                    in_=xparam[b, kc * 128:(kc + 1) * 128,
                                               c0:c0 + SBLK])
                                for nh in range(2):
                                    nc.tensor.matmul(
                                        ps[:, nh * 512:(nh + 1) * 512],
                                        w_t[:, kc, :],
                                        xt[:, nh * 512:(nh + 1) * 512],
                                        start=(kc == 0), stop=(kc == KC - 1))
                            pss.append(ps)

                        def var_block(i):
                            ps = pss[i]
                            qc = qcp.tile([128, SBLK], BF16, tag="qc",
                                          name="qc")
                            nc.scalar.copy(qc, ps[:, :])
                            qcs.append(qc)
                            sq = p1sb.tile([128, SBLK], F32R, tag="sq")
                            nc.vector.tensor_mul(sq, qc, qc)
                            var_ps = p1ps.tile([128, SBLK], F32, tag="ps",
                                               name="var_ps")
                            for nh in range(2):
                                nc.tensor.matmul(
                                    var_ps[0:2, nh * 512:(nh + 1) * 512],
                                    blockones,
                                    sq[:, nh * 512:(nh + 1) * 512],
                                    start=True, stop=True)
                            nc.scalar.activation(
                                u_all[:, i, :], var_ps[0:2, :], AF.Copy,
                                bias=EPS, scale=1.0 / HD)

                        mm_block(0)
                        for i in range(1, NBLK):
                            mm_block(i)
                            var_block(i - 1)
                        var_block(NBLK - 1)
                        return u_all, qcs

                    def qk_sweep23(u_all, qcs, g_t, b_t, dstT):
                        """Full-lane rsqrt, then broadcast + apply."""
                        nlane = NBLK * SBLK * 2 // 128
                        u128 = p1st.tile([128, nlane], F32, tag="u128")
                        nc.gpsimd.dma_start(
                            out=u128[:, :],
                            in_=u_all.rearrange("p i n -> p (i n)"))
                        r128 = _rsqrt_dve(nc, p1st, u128, 128, nlane, "ln")
                        rstd2 = p1st.tile([2, NBLK, SBLK], F32R, tag="rstd2")
                        nc.gpsimd.dma_start(
                            out=rstd2.rearrange("p i n -> p (i n)"),
                            in_=r128[:, :])
                        for i in range(NBLK):
                            b, sb = divmod(i, NSB)
                            c0 = sb * SBLK
                            rb_ps = p1ps.tile([128, SBLK], F32, tag="ps",
                                              name="rb_ps")
                            for nh in range(2):
                                nc.tensor.matmul(
                                    rb_ps[:, nh * 512:(nh + 1) * 512], bc2,
                                    rstd2[:, i, nh * 512:(nh + 1) * 512],
                                    start=True, stop=True)
                            t1 = p1sb.tile([128, SBLK], F32, tag="t1")
                            nc.vector.scalar_tensor_tensor(
                                out=t1, in0=qcs[i], scalar=g_t,
                                in1=rb_ps[:, :],
                                op0=OP.mult, op1=OP.mult)
                            nc.vector.tensor_scalar(
                                out=dstT[:, b, c0:c0 + SBLK], in0=t1,
                                scalar1=b_t, scalar2=None, op0=OP.add)

                    def proj_v():
                        for b in range(B):
                            for sb in range(NSB):
                                c0 = sb * SBLK
                                ps = p1ps.tile([128, SBLK], F32, tag="ps",
                                               name="ps")
                                for kc in range(KC):
                                    xt = xs.tile([128, SBLK], BF16, tag="x")
                                    nc.sync.dma_start(
                                        out=xt,
                                        in_=xvT[b, kc * 128:(kc + 1) * 128,
                                                c0:c0 + SBLK])
                                    for nh in range(2):
                                        nc.tensor.matmul(
                                            ps[:, nh * 512:(nh + 1) * 512],
                                            wv_t[:, kc, :],
                                            xt[:, nh * 512:(nh + 1) * 512],
                                            start=(kc == 0),
                                            stop=(kc == KC - 1))
                                vts = p1sb.tile([128, SBLK], BF16, tag="vts")
                                nc.scalar.copy(vts, ps[:, :])
                                for q4 in range(2):
                                    tp = tpps.tile([128, 512], BF16, tag="tp")
                                    for j in range(4):
                                        nc.tensor.transpose(
                                            tp[:, j * 128:(j + 1) * 128],
                                            vts[:, (q4 * 4 + j) * 128:
                                                (q4 * 4 + j + 1) * 128],
                                            ident_bf[:, :])
                                    fc0 = sb * 8 + q4 * 4
                                    nc.vector.tensor_copy(
                                        vx[:, b, fc0:fc0 + 4, :, 1:HD + 1],
                                        tp[:, :].rearrange(
                                            "p (j h d) -> p j h d",
                                            h=HPC, d=HD))

                    uk, qck = qk_sweep1(xkT, wk_t)
                    uq, qcq = qk_sweep1(xqT, wq_t)
                    qk_sweep23(uk, qck, kg_t, kb_t, kT)
                    proj_v()
                    qk_sweep23(uq, qcq, qg_t, qb_t, qT)

                # ================= phase 2: attention =================
                with (
                    tc.tile_pool(name="att", bufs=3) as att,
                    tc.tile_pool(name="attps", bufs=2, space="PSUM") as attps,
                    tc.tile_pool(name="ctxps", bufs=2, space="PSUM") as ctxps,
                ):
                    def normalize(blk, ctx_ps):
                        b, ab = divmod(blk, NAB)
                        for hh in range(2):
                            den = att.tile([1, ABLK], F32R, tag="den")
                            with nc.allow_low_precision(
                                    reason="fp32r is 32-bit storage"):
                                nc.vector.reciprocal(den, ctx_ps[hh][0:1, :])
                            bc_ps = attps.tile([128, 2 * ABLK], F32,
                                               tag="sp", name="bc_ps")
                            nc.tensor.matmul(
                                bc_ps[0:HD + 1, 0:ABLK],
                                ones_row[:, 0:HD + 1], den,
                                start=True, stop=True)
                            rbc = att.tile([HD + 1, ABLK], F32, tag="rbc")
                            nc.vector.tensor_copy(rbc, bc_ps[0:HD + 1, 0:ABLK])
                            tmp = att.tile([HD + 1, ABLK], BF16, tag="ctmp")
                            nc.vector.tensor_mul(tmp, ctx_ps[hh][:, :], rbc)
                            nc.gpsimd.dma_start(
                                out=a2a_in[blk][hh * 64:(hh + 1) * 64, :],
                                in_=tmp[1:HD + 1, :])

                    prev = None
                    for blk in range(B * NAB):
                        b, ab = divmod(blk, NAB)
                        s0 = ab * ABLK
                        ctx_ps = [
                            ctxps.tile([HD + 1, ABLK], F32, tag="ctxA",
                                       name="ctxA"),
                            ctxps.tile([HD + 1, ABLK], F32, tag="ctxB",
                                       name="ctxB"),
                        ]
                        for sc in range(FCN // 2):
                            sp = [attps.tile([128, 2 * ABLK], F32,
                                             tag="sp", name="sp")
                                  for _ in range(2)]
                            for cc in range(2):
                                fc = sc * 2 + cc
                                for hh in range(2):
                                    nc.tensor.matmul(
                                        sp[hh][:, cc * ABLK:(cc + 1) * ABLK],
                                        kT[hh * 64:(hh + 1) * 64, b,
                                           fc * 128:(fc + 1) * 128],
                                        qT[hh * 64:(hh + 1) * 64, b,
                                           s0:s0 + ABLK],
                                        start=True, stop=True,
                                        tile_position=(hh * 64, 0))
                            pt = []
                            for hh in range(2):
                                p = att.tile([128, 2 * ABLK], BF16,
                                             tag="pt", name="pt")
                                nc.scalar.activation(p, sp[hh][:, :], AF.Exp)
                                pt.append(p)
                            for cc in range(2):
                                fc = sc * 2 + cc
                                for hh in range(2):
                                    nc.tensor.matmul(
                                        ctx_ps[hh][:, :],
                                        vx[:, b, fc, hh, :],
                                        pt[hh][:, cc * ABLK:(cc + 1) * ABLK],
                                        start=(fc == 0),
                                        stop=(fc == FCN - 1))
                            if sc == 0 and prev is not None:
                                normalize(blk - 1, prev)
                        prev = ctx_ps
                    normalize(B * NAB - 1, prev)

                # Wproj load: deferred so its 4 MB DMA rides P2's idle
                # DMA window instead of competing with the x-streams
                nc.gpsimd.dma_start(
                    out=wp_t,
                    in_=wp.rearrange("(kc p) n -> p kc n", p=128).bitcast(F32R))

                # ================= all-to-all re-shard =================
                nc.gpsimd.collective_compute(
                    "AllToAll", mybir.AluOpType.bypass,
                    replica_groups=[list(range(NCORES))],
                    ins=[a2a_in.opt()], outs=[a2a_out.opt()],
                )
                nc.sync.dma_start(
                    out=ctxF, in_=a2a_out.rearrange("j p r -> p j r"))

                # ============ phase 3: out-LN + projection ============
                with (
                    tc.tile_pool(name="p3", bufs=2) as p3,
                    tc.tile_pool(name="p3w", bufs=1) as p3w,
                    tc.tile_pool(name="p3s", bufs=1) as p3s,
                    tc.tile_pool(name="st_ps", bufs=1, space="PSUM") as st_ps,
                    tc.tile_pool(name="o_ps", bufs=2, space="PSUM") as o_ps,
                ):
                    # bias vector broadcast (beta @ Wproj), computed once
                    bvb_ps = o_ps.tile([128, D], F32, tag="o", name="bvb_ps")
                    for nh in range(2):
                        nc.tensor.matmul(
                            bvb_ps[:, nh * 512:(nh + 1) * 512], ones_row,
                            bv_t[:, nh * 512:(nh + 1) * 512],
                            start=True, stop=True)
                    bvb = p3w.tile([128, D], F32, tag="bvb_sb")
                    nc.vector.tensor_copy(bvb, bvb_ps[:, :])

                    sum_ps = st_ps.tile([1, OUT_ROWS], F32, tag="sum")
                    for kc in range(KC):
                        nc.tensor.matmul(
                            sum_ps[:, :], ones_col_bf, ctxF[:, kc, :],
                            start=(kc == 0), stop=(kc == KC - 1))
                    sq_ps = st_ps.tile([1, OUT_ROWS], F32, tag="sq")
                    for kc in range(KC):
                        sq = p3.tile([128, OUT_ROWS], F32R, tag="sq")
                        nc.vector.tensor_mul(sq, ctxF[:, kc, :],
                                             ctxF[:, kc, :])
                        nc.tensor.matmul(
                            sq_ps[:, :], ones_col_r, sq,
                            start=(kc == 0), stop=(kc == KC - 1))
                    # copy sums to SBUF, reshape to 128 lanes for the
                    # small-stat math (DVE lanes are per-partition);
                    # DMAs pair elements in linearized AP order, so a
                    # (1,512) <-> (128,4) move is just a reshape.
                    ssb = p3s.tile([1, 2, OUT_ROWS], F32, tag="ssb")
                    nc.scalar.copy(ssb[:, 0, :], sum_ps[:, :])
                    nc.scalar.copy(ssb[:, 1, :], sq_ps[:, :])
                    s_sum = p3s.tile([128, 4], F32, tag="s_sum")
                    nc.scalar.dma_start(out=s_sum[:, :], in_=ssb[:, 0, :])
                    s_sq = p3s.tile([128, 4], F32, tag="s_sq")
                    nc.scalar.dma_start(out=s_sq[:, :], in_=ssb[:, 1, :])
                    mean8 = p3s.tile([128, 4], F32, tag="mean8")
                    nc.vector.tensor_scalar_mul(mean8, s_sum[:, :], 1.0 / D)
                    m28 = p3s.tile([128, 4], F32, tag="m28")
                    nc.vector.tensor_mul(m28, mean8, mean8)
                    u8 = p3s.tile([128, 4], F32, tag="u8")
                    nc.vector.tensor_scalar(
                        out=u8, in0=s_sq[:, :], scalar1=1.0 / D, scalar2=EPS,
                        op0=OP.mult, op1=OP.add)
                    nc.vector.tensor_sub(u8, u8, m28)
                    r8 = _rsqrt_dve(nc, p3s, u8, 128, 4, "oln")
                    negm8 = p3s.tile([128, 4], F32R, tag="negm8")
                    with nc.allow_low_precision(
                            reason="fp32r is 32-bit storage"):
                        nc.vector.tensor_mul(negm8, mean8, r8.bitcast(F32))
                        nc.vector.tensor_scalar_mul(negm8, negm8, -1.0)
                    rstd = p3s.tile([1, OUT_ROWS], F32R, tag="rstd")
                    nc.scalar.dma_start(out=rstd[:, :], in_=r8[:, :])
                    negm = p3s.tile([1, OUT_ROWS], F32R, tag="negm")
                    nc.scalar.dma_start(out=negm[:, :], in_=negm8[:, :])
                    rstd_ps = st_ps.tile([128, OUT_ROWS], F32, tag="rstd_ps")
                    nc.tensor.matmul(
                        rstd_ps[:, :], ones_row, rstd,
                        start=True, stop=True)
                    negm_ps = st_ps.tile([128, OUT_ROWS], F32, tag="negm_ps")
                    nc.tensor.matmul(
                        negm_ps[:, :], ones_row, negm,
                        start=True, stop=True)

                    ctxn = p3w.tile([128, KC, OUT_ROWS], F32R, tag="ctxn")
                    for kc in range(KC):
                        t = p3.tile([128, OUT_ROWS], F32, tag="lnt")
                        nc.vector.tensor_mul(t, ctxF[:, kc, :], rstd_ps[:, :])
                        with nc.allow_low_precision(
                                reason="fp32r is 32-bit storage"):
                            nc.vector.tensor_add(ctxn[:, kc, :], t,
                                                 negm_ps[:, :])

                    for m in range(OUT_ROWS // 128):
                        ps = o_ps.tile([128, D], F32, tag="o")
                        for n in range(2):
                            for kc in range(KC):
                                nc.tensor.matmul(
                                    ps[:, n * 512:(n + 1) * 512],
                                    ctxn[:, kc, m * 128:(m + 1) * 128],
                                    wp_t[:, kc, n * 512:(n + 1) * 512],
                                    start=(kc == 0), stop=(kc == KC - 1))
                        o_sb = p3.tile([128, D], F32, tag="osb")
                        nc.vector.tensor_add(o_sb, ps[:, :], bvb)
                        nc.sync.dma_start(
                            out=out[m * 128:(m + 1) * 128, :], in_=o_sb)
    nc.finalize()
    return nc


def _center_per_head(W):
    Wc = np.asarray(W, np.float32).reshape(D, H, HD)
    return (Wc - Wc.mean(axis=2, keepdims=True)).reshape(D, D)


def make_in_maps(x_q, x_k, x_v, Wq, Wk, Wv, Wproj,
                 q_gamma, q_beta, k_gamma, k_beta, out_gamma, out_beta):
    """Host-side prep: transpose inputs, center + slice weights, fold scale,
    fold out-LN gamma/beta into Wproj / a bias vector."""
    bf = ml_dtypes.bfloat16
    scale = np.float32(HD ** -0.5)
    xqT = np.ascontiguousarray(
        np.asarray(x_q, np.float32).transpose(0, 2, 1)).astype(bf)
    xkT = np.ascontiguousarray(
        np.asarray(x_k, np.float32).transpose(0, 2, 1)).astype(bf)
    xvT = np.ascontiguousarray(
        np.asarray(x_v, np.float32).transpose(0, 2, 1)).astype(bf)
    WqC = _center_per_head(Wq)
    WkC = _center_per_head(Wk)
    Wv = np.ascontiguousarray(np.asarray(Wv, np.float32))
    Wp = np.asarray(Wproj, np.float32)
    og = np.asarray(out_gamma, np.float32)
    ob = np.asarray(out_beta, np.float32)
    Wp2 = np.ascontiguousarray(Wp * og[:, None])
    bvec = ob @ Wp
    qg2 = np.tile(np.asarray(q_gamma, np.float32) * scale, HPC)
    qb2 = np.tile(np.asarray(q_beta, np.float32) * scale, HPC)
    kg2 = np.tile(np.asarray(k_gamma, np.float32), HPC)
    kb2 = np.tile(np.asarray(k_beta, np.float32), HPC)
    bo2 = np.zeros((128, 2), np.float32)
    bo2[0:64, 0] = 1.0
    bo2[64:128, 1] = 1.0
    cb2 = np.zeros((2, 128), np.float32)
    cb2[0, 0:64] = 1.0
    cb2[1, 64:128] = 1.0
    idp = np.eye(128, dtype=np.float32).astype(bf)
    in_maps = []
    for c in range(NCORES):
        dc = slice(DPC * c, DPC * (c + 1))
        in_maps.append({
            "xqT": xqT, "xkT": xkT, "xvT": xvT,
            "wq": np.ascontiguousarray(WqC[:, dc]).astype(bf),
            "wk": np.ascontiguousarray(WkC[:, dc]).astype(bf),
            "wv": np.ascontiguousarray(Wv[:, dc]).astype(bf),
            "wp": Wp2, "bv": bvec,
            "qg": qg2, "qb": qb2, "kg": kg2, "kb": kb2,
            "bo2": bo2, "cb2": cb2, "idp": idp,
        })
    return in_maps


_NC_CACHE = None


def _get_nc():
    global _NC_CACHE
    if _NC_CACHE is None:
        _NC_CACHE = build(1)
    return _NC_CACHE


def kernel(x_q, x_k, x_v, Wq, Wk, Wv, Wproj,
           q_gamma, q_beta, k_gamma, k_beta, out_gamma, out_beta,
           _trace=False):
    in_maps = make_in_maps(x_q, x_k, x_v, Wq, Wk, Wv, Wproj,
                           q_gamma, q_beta, k_gamma, k_beta,
                           out_gamma, out_beta)
    nc = _get_nc()
    res = bass_utils.run_bass_kernel_spmd(
        nc, in_maps, list(range(NCORES)), trace=_trace)
    full = np.empty((B, S, D), dtype=np.float32)
    for c in range(NCORES):
        b, r = c // 4, c % 4
        full[b, r * OUT_ROWS:(r + 1) * OUT_ROWS, :] = res.results[c]["out"]
    if _trace:
        return full, res
    return full


# revision 6
# speedup vs baseline: 1.2037x; 1.1508x over previous
"""Multi-head attention with QK-LayerNorm on 8 Trainium2 NeuronCores.

Problem: B=2, S=F=2048, D=1024, H=16, HD=64 (fp32).
    q = LN_head(x_q @ Wq) * HD^-0.5 ; k = LN_head(x_k @ Wk) ; v = x_v @ Wv
    ctx = softmax(q k^T) v ; out = LN(ctx) @ Wproj

Sharding (tensor-parallel over heads): core c owns heads [2c, 2c+1]
(d-columns [128c:128c+128]) for BOTH batches.  Each core receives the
full transposed inputs x^T (both batches, bf16) plus its 128-column
weight slices, computes q/k/v projections directly in transposed
layout, runs attention for its 2 heads over all 4096 query rows, and
normalizes ctx by the softmax denominator.  A single 8-core AllToAll
(1 MB, bf16) re-shards ctx from head-split to row-split; core c ends
with ctx rows [batch c//4, 512*(c%4):...] x full D, then does the
output LayerNorm + projection locally.  No AllGather of k/v is needed
because every core computes its own heads' k/v from the full input.

QK-LN mean is folded into the weights on the host (per-head column
centering makes the projection mean-free), so only the variance is
computed on device (PE block-ones matmul + DVE Newton rsqrt; the ACT
engine runs exp exclusively - one table set, no switches).  Out-LN
gamma/beta are folded into Wproj / a bias vector on the host.
"""

import numpy as np
import ml_dtypes

import concourse.bass as bass
import concourse.mybir as mybir
import concourse.tile as tile
from concourse import bacc, bass_utils

F32 = mybir.dt.float32
F32R = mybir.dt.float32r
BF16 = mybir.dt.bfloat16
I32 = mybir.dt.int32
AF = mybir.ActivationFunctionType
OP = mybir.AluOpType

B, S, F, D, H, HD = 2, 2048, 2048, 1024, 16, 64
EPS = 1e-5
NCORES = 8
HPC = 2                    # heads per core
DPC = HPC * HD             # 128 d-columns per core
KC = D // 128              # 8 contraction chunks
SBLK = 1024                # projection s-block
NSB = S // SBLK            # 2
NBLK = B * NSB             # 4 projection blocks per tensor
ABLK = 512                 # attention s-block
NAB = S // ABLK            # 4
FCN = F // 128             # 16 f-chunks
OUT_ROWS = B * S // NCORES  # 512 output rows per core
MAGIC = 0x5F3759DF


def _rsqrt_dve(nc, pool, u, nparts, ncols, tag):
    """y = 1/sqrt(u) via bit trick + 2 Newton steps, all on DVE.

    u: SBUF F32 AP (nparts, ncols), strictly positive. Returns F32R tile.
    """
    yt = pool.tile([nparts, ncols], F32, tag=f"{tag}_y")
    nc.vector.tensor_scalar(
        out=yt.bitcast(I32), in0=u.bitcast(I32),
        scalar1=1, scalar2=None, op0=OP.logical_shift_right)
    nc.vector.tensor_scalar(
        out=yt.bitcast(I32), in0=yt.bitcast(I32),
        scalar1=-1, scalar2=MAGIC, op0=OP.mult, op1=OP.add)
    yr = pool.tile([nparts, ncols], F32R, tag=f"{tag}_yr")
    for it in range(2):
        t2 = pool.tile([nparts, ncols], F32, tag=f"{tag}_t")
        nc.vector.tensor_mul(t2, yt, yt)
        nc.vector.tensor_mul(t2, t2, u)
        nc.vector.tensor_scalar(
            out=t2, in0=t2, scalar1=-0.5, scalar2=1.5, op0=OP.mult, op1=OP.add)
        if it == 0:
            nc.vector.tensor_mul(yt, yt, t2)
        else:
            with nc.allow_low_precision(reason="fp32r is 32-bit storage"):
                nc.vector.tensor_mul(yr, yt, t2)
    return yr


def build(n_repeat=1):
    nc = bacc.Bacc(None, target_bir_lowering=False)

    xqT = nc.declare_dram_parameter("xqT", [B, D, S], BF16, isOutput=False)
    xkT = nc.declare_dram_parameter("xkT", [B, D, S], BF16, isOutput=False)
    xvT = nc.declare_dram_parameter("xvT", [B, D, S], BF16, isOutput=False)
    wq = nc.declare_dram_parameter("wq", [D, DPC], BF16, isOutput=False)
    wk = nc.declare_dram_parameter("wk", [D, DPC], BF16, isOutput=False)
    wv = nc.declare_dram_parameter("wv", [D, DPC], BF16, isOutput=False)
    wp = nc.declare_dram_parameter("wp", [D, D], F32, isOutput=False)
    bv = nc.declare_dram_parameter("bv", [D], F32, isOutput=False)
    qg = nc.declare_dram_parameter("qg", [DPC], F32, isOutput=False)  # *scale
    qb = nc.declare_dram_parameter("qb", [DPC], F32, isOutput=False)  # *scale
    kg = nc.declare_dram_parameter("kg", [DPC], F32, isOutput=False)
    kb = nc.declare_dram_parameter("kb", [DPC], F32, isOutput=False)
    bo2 = nc.declare_dram_parameter("bo2", [128, 2], F32, isOutput=False)
    idp = nc.declare_dram_parameter("idp", [128, 128], BF16, isOutput=False)
    cb2 = nc.declare_dram_parameter("cb2", [2, 128], F32, isOutput=False)
    out = nc.declare_dram_parameter("out", [OUT_ROWS, D], F32, isOutput=True)

    with tile.TileContext(nc) as tc:
        with (
            tc.tile_pool(name="const", bufs=1) as const,
            tc.tile_pool(name="persist", bufs=1) as persist,
            tc.tile_pool(name="dram", bufs=1, space="DRAM") as dram,
        ):
            # ---- constants ----
            ident_bf = const.tile([128, 128], BF16)
            nc.gpsimd.dma_start(out=ident_bf, in_=idp[:, :])
            ones_row_f = const.tile([1, 128], F32)
            nc.vector.memset(ones_row_f, 1.0)
            ones_row = const.tile([1, 128], F32R)
            nc.vector.tensor_copy(ones_row, ones_row_f)
            ones_col_bf = const.tile([128, 1], BF16)
            nc.vector.memset(ones_col_bf, 1.0)
            ones_col_f = const.tile([128, 1], F32)
            nc.vector.memset(ones_col_f, 1.0)
            ones_col_r = const.tile([128, 1], F32R)
            nc.vector.tensor_copy(ones_col_r, ones_col_f)
            # block-ones (128, 2): col h = 1 on partitions [64h, 64h+64)
            blockones = const.tile([128, 2], F32R)
            nc.gpsimd.dma_start(out=blockones, in_=bo2[:, :].bitcast(F32R))
            # bcast (2, 128): row h = 1 on cols [64h, 64h+64)
            bc2 = const.tile([2, 128], F32R)
            nc.gpsimd.dma_start(out=bc2, in_=cb2[:, :].bitcast(F32R))

            def gb_tile(param):
                t = const.tile([128, 1], F32, tag=f"gb_{param.name}")
                nc.gpsimd.dma_start(
                    out=t, in_=param.rearrange("(p one) -> p one", one=1))
                return t

            qg_t, qb_t = gb_tile(qg), gb_tile(qb)
            kg_t, kb_t = gb_tile(kg), gb_tile(kb)
            bv_t = const.tile([1, D], F32R)
            nc.gpsimd.dma_start(
                out=bv_t,
                in_=bv.rearrange("(one n) -> one n", one=1).bitcast(F32R))

            for _rep in range(n_repeat):
                # ---- persistent SBUF ----
                qT = persist.tile([128, B, S], BF16, tag="qT")
                kT = persist.tile([128, B, S], BF16, tag="kT")
                vx = persist.tile([128, B, FCN, HPC, HD + 1], BF16, tag="vx")
                ctxF = persist.tile([128, KC, 2, OUT_ROWS // 2], BF16,
                                    tag="ctxF")
                nc.vector.memset(vx[:, :, :, :, 0:1], 1.0)

                def load_w(param, tag):
                    t = persist.tile([128, KC, DPC], BF16, tag=tag)
                    nc.sync.dma_start(
                        out=t, in_=param.rearrange("(kc p) n -> p kc n", p=128))
                    return t

                wk_t = load_w(wk, "wk")
                wq_t = load_w(wq, "wq")
                wv_t = load_w(wv, "wv")
                wp_t = persist.tile([128, KC, D], F32R, tag="wp")

                # ---- DRAM all-to-all buffers (two row-halves) ----
                HR = OUT_ROWS // 2
                a2a_in = [dram.tile([NCORES, 128, HR], BF16,
                                    tag=f"a2ai{h}", name="a2ai")
                          for h in range(2)]
                a2a_out = [dram.tile([NCORES, 128, HR], BF16,
                                     tag=f"a2ao{h}", name="a2ao")
                           for h in range(2)]

                # ================= phase 1: projections =================
                with (
                    tc.tile_pool(name="xs", bufs=10) as xs,
                    tc.tile_pool(name="qcp", bufs=8) as qcp,
                    tc.tile_pool(name="p1sb", bufs=2) as p1sb,
                    tc.tile_pool(name="p1st", bufs=1) as p1st,
                    tc.tile_pool(name="p1u", bufs=2) as p1u,
                    tc.tile_pool(name="p1ps", bufs=3, space="PSUM") as p1ps,
                    tc.tile_pool(name="tpps", bufs=2, space="PSUM") as tpps,
                ):
                    def qk_sweep1(xparam, w_t):
                        """Dense proj matmuls + variance, one-block pipeline."""
                        u_all = p1u.tile([2, NBLK, SBLK], F32, tag="u")
                        qcs, pss = [], []

                        def mm_block(i):
                            b, sb = divmod(i, NSB)
                            c0 = sb * SBLK
                            ps = p1ps.tile([128, SBLK], F32, tag="ps",
                                           name="ps")
                            for kc in range(KC):
                                xt = xs.tile([128, SBLK], BF16, tag="x")
                                nc.sync.dma_start(
                                    out=xt,
                                    in_=xparam[b, kc * 128:(kc + 1) * 128,
                                               c0:c0 + SBLK])
                                for nh in range(2):
                                    nc.tensor.matmul(
                                        ps[:, nh * 512:(nh + 1) * 512],
                                        w_t[:, kc, :],
                                        xt[:, nh * 512:(nh + 1) * 512],
                                        start=(kc == 0), stop=(kc == KC - 1))
                            pss.append(ps)

                        def var_block(i):
                            ps = pss[i]
                            qc = qcp.tile([128, SBLK], BF16, tag="qc",
                                          name="qc")
                            nc.scalar.copy(qc, ps[:, :])
                            qcs.append(qc)
                            sq = p1sb.tile([128, SBLK], F32R, tag="sq")
                            nc.vector.tensor_mul(sq, qc, qc)
                            var_ps = p1ps.tile([128, SBLK], F32, tag="ps",
                                               name="var_ps")
                            for nh in range(2):
                                nc.tensor.matmul(
                                    var_ps[0:2, nh * 512:(nh + 1) * 512],
                                    blockones,
                                    sq[:, nh * 512:(nh + 1) * 512],
                                    start=True, stop=True)
                            nc.scalar.activation(
                                u_all[:, i, :], var_ps[0:2, :], AF.Copy,
                                bias=EPS, scale=1.0 / HD)

                        mm_block(0)
                        for i in range(1, NBLK):
                            mm_block(i)
                            var_block(i - 1)
                        var_block(NBLK - 1)
                        return u_all, qcs

                    def qk_sweep23(u_all, qcs, g_t, b_t, dstT):
                        """Full-lane rsqrt, then broadcast + apply."""
                        nlane = NBLK * SBLK * 2 // 128
                        u128 = p1st.tile([128, nlane], F32, tag="u128")
                        nc.gpsimd.dma_start(
                            out=u128[:, :],
                            in_=u_all.rearrange("p i n -> p (i n)"))
                        r128 = _rsqrt_dve(nc, p1st, u128, 128, nlane, "ln")
                        rstd2 = p1st.tile([2, NBLK, SBLK], F32R, tag="rstd2")
                        nc.gpsimd.dma_start(
                            out=rstd2.rearrange("p i n -> p (i n)"),
                            in_=r128[:, :])
                        for i in range(NBLK):
                            b, sb = divmod(i, NSB)
                            c0 = sb * SBLK
                            rb_ps = p1ps.tile([128, SBLK], F32, tag="ps",
                                              name="rb_ps")
                            for nh in range(2):
                                nc.tensor.matmul(
                                    rb_ps[:, nh * 512:(nh + 1) * 512], bc2,
                                    rstd2[:, i, nh * 512:(nh + 1) * 512],
                                    start=True, stop=True)
                            t1 = p1sb.tile([128, SBLK], F32, tag="t1")
                            nc.vector.scalar_tensor_tensor(
                                out=t1, in0=qcs[i], scalar=g_t,
                                in1=rb_ps[:, :],
                                op0=OP.mult, op1=OP.mult)
                            nc.vector.tensor_scalar(
                                out=dstT[:, b, c0:c0 + SBLK], in0=t1,
                                scalar1=b_t, scalar2=None, op0=OP.add)

                    def proj_v():
                        for b in range(B):
                            for sb in range(NSB):
                                c0 = sb * SBLK
                                ps = p1ps.tile([128, SBLK], F32, tag="ps",
                                               name="ps")
                                for kc in range(KC):
                                    xt = xs.tile([128, SBLK], BF16, tag="x")
                                    nc.sync.dma_start(
                                        out=xt,
                                        in_=xvT[b, kc * 128:(kc + 1) * 128,
                                                c0:c0 + SBLK])
                                    for nh in range(2):
                                        nc.tensor.matmul(
                                            ps[:, nh * 512:(nh + 1) * 512],
                                            wv_t[:, kc, :],
                                            xt[:, nh * 512:(nh + 1) * 512],
                                            start=(kc == 0),
                                            stop=(kc == KC - 1))
                                vts = p1sb.tile([128, SBLK], BF16, tag="vts")
                                nc.scalar.copy(vts, ps[:, :])
                                for q4 in range(2):
                                    tp = tpps.tile([128, 512], BF16, tag="tp")
                                    for j in range(4):
                                        nc.tensor.transpose(
                                            tp[:, j * 128:(j + 1) * 128],
                                            vts[:, (q4 * 4 + j) * 128:
                                                (q4 * 4 + j + 1) * 128],
                                            ident_bf[:, :])
                                    fc0 = sb * 8 + q4 * 4
                                    nc.vector.tensor_copy(
                                        vx[:, b, fc0:fc0 + 4, :, 1:HD + 1],
                                        tp[:, :].rearrange(
                                            "p (j h d) -> p j h d",
                                            h=HPC, d=HD))

                    uk, qck = qk_sweep1(xkT, wk_t)
                    uq, qcq = qk_sweep1(xqT, wq_t)
                    qk_sweep23(uk, qck, kg_t, kb_t, kT)
                    proj_v()
                    qk_sweep23(uq, qcq, qg_t, qb_t, qT)

                # ================= phase 2: attention =================
                with (
                    tc.tile_pool(name="att", bufs=3) as att,
                    tc.tile_pool(name="attps", bufs=2, space="PSUM") as attps,
                    tc.tile_pool(name="ctxps", bufs=2, space="PSUM") as ctxps,
                ):
                    def emit_cc(h):
                        nc.gpsimd.collective_compute(
                            "AllToAll", mybir.AluOpType.bypass,
                            replica_groups=[list(range(NCORES))],
                            ins=[a2a_in[h].opt()], outs=[a2a_out[h].opt()],
                        )
                        nc.sync.dma_start(
                            out=ctxF[:, :, h, :],
                            in_=a2a_out[h].rearrange("j p r -> p j r"))

                    HB = 256                 # rows per half-block
                    def normalize(blk, ctx_ps):
                        h, j = divmod(blk, NCORES)
                        for hh in range(2):
                            den = att.tile([1, HB], F32R, tag="den")
                            with nc.allow_low_precision(
                                    reason="fp32r is 32-bit storage"):
                                nc.vector.reciprocal(den, ctx_ps[hh][0:1, :])
                            bc_ps = attps.tile([128, 4 * HB], F32,
                                               tag="sp", name="bc_ps")
                            nc.tensor.matmul(
                                bc_ps[0:HD + 1, 0:HB],
                                ones_row[:, 0:HD + 1], den,
                                start=True, stop=True)
                            rbc = att.tile([HD + 1, HB], F32, tag="rbc")
                            nc.vector.tensor_copy(rbc, bc_ps[0:HD + 1, 0:HB])
                            tmp = att.tile([HD + 1, HB], BF16, tag="ctmp")
                            nc.vector.tensor_mul(tmp, ctx_ps[hh][:, :], rbc)
                            nc.gpsimd.dma_start(
                                out=a2a_in[h][j][hh * 64:(hh + 1) * 64, :],
                                in_=tmp[1:HD + 1, :])
                        if blk == NCORES - 1:
                            emit_cc(0)

                    prev = None
                    for blk in range(2 * NCORES):
                        h, j = divmod(blk, NCORES)
                        b, rj = divmod(j, 4)
                        s0 = rj * ABLK + h * HB
                        ctx_ps = [
                            ctxps.tile([HD + 1, HB], F32, tag="ctxA",
                                       name="ctxA"),
                            ctxps.tile([HD + 1, HB], F32, tag="ctxB",
                                       name="ctxB"),
                        ]
                        stage = att.tile([128, FCN // 4, 4 * HB], BF16,
                                         tag="stage", name="stage")
                        pts = []
                        for sc in range(FCN // 4):
                            sp = [attps.tile([128, 4 * HB], F32,
                                             tag="sp", name="sp")
                                  for _ in range(2)]
                            for cc in range(4):
                                fc = sc * 4 + cc
                                for hh in range(2):
                                    nc.tensor.matmul(
                                        sp[hh][:, cc * HB:(cc + 1) * HB],
                                        kT[hh * 64:(hh + 1) * 64, b,
                                           fc * 128:(fc + 1) * 128],
                                        qT[hh * 64:(hh + 1) * 64, b,
                                           s0:s0 + HB],
                                        start=True, stop=True,
                                        tile_position=(hh * 64, 0))
                            p = att.tile([128, 4 * HB], BF16,
                                         tag="pt", name="pt")
                            nc.scalar.activation(p, sp[0][:, :], AF.Exp)
                            pts.append(p)
                            nc.vector.tensor_copy(stage[:, sc, :],
                                                  sp[1][:, :])
                            for cc in range(4):
                                fc = sc * 4 + cc
                                nc.tensor.matmul(
                                    ctx_ps[0][:, :],
                                    vx[:, b, fc, 0, :],
                                    p[:, cc * HB:(cc + 1) * HB],
                                    start=(fc == 0),
                                    stop=(fc == FCN - 1))
                            if sc == 0 and prev is not None:
                                normalize(blk - 1, prev)
                        ptb = att.tile([128, FCN // 4, 4 * HB], BF16,
                                       tag="ptb", name="ptb")
                        nc.scalar.activation(
                            ptb.rearrange("p a c -> p (a c)"),
                            stage.rearrange("p a c -> p (a c)"), AF.Exp)
                        for sc in range(FCN // 4):
                            for cc in range(4):
                                fc = sc * 4 + cc
                                nc.tensor.matmul(
                                    ctx_ps[1][:, :],
                                    vx[:, b, fc, 1, :],
                                    ptb[:, sc, cc * HB:(cc + 1) * HB],
                                    start=(fc == 0),
                                    stop=(fc == FCN - 1))
                        prev = ctx_ps
                    normalize(2 * NCORES - 1, prev)

                # Wproj load: deferred so its 4 MB DMA rides P2's idle
                # DMA window instead of competing with the x-streams
                nc.gpsimd.dma_start(
                    out=wp_t,
                    in_=wp.rearrange("(kc p) n -> p kc n", p=128).bitcast(F32R))

                # ================= all-to-all re-shard (2nd half) ======
                emit_cc(1)

                # ============ phase 3: out-LN + projection ============
                with (
                    tc.tile_pool(name="p3", bufs=2) as p3,
                    tc.tile_pool(name="p3w", bufs=1) as p3w,
                    tc.tile_pool(name="p3s", bufs=1) as p3s,
                    tc.tile_pool(name="st_ps", bufs=1, space="PSUM") as st_ps,
                    tc.tile_pool(name="o_ps", bufs=2, space="PSUM") as o_ps,
                ):
                    # bias vector broadcast (beta @ Wproj), computed once
                    bvb_ps = o_ps.tile([128, D], F32, tag="o", name="bvb_ps")
                    for nh in range(2):
                        nc.tensor.matmul(
                            bvb_ps[:, nh * 512:(nh + 1) * 512], ones_row,
                            bv_t[:, nh * 512:(nh + 1) * 512],
                            start=True, stop=True)
                    bvb = p3w.tile([128, D], F32, tag="bvb_sb")
                    nc.vector.tensor_copy(bvb, bvb_ps[:, :])

                    HR = OUT_ROWS // 2
                    for h in range(2):
                        sum_ps = st_ps.tile([1, HR], F32, tag="sum",
                                            name="sum_ps")
                        for kc in range(KC):
                            nc.tensor.matmul(
                                sum_ps[:, :], ones_col_bf, ctxF[:, kc, h, :],
                                start=(kc == 0), stop=(kc == KC - 1))
                        sq_ps = st_ps.tile([1, HR], F32, tag="sq",
                                           name="sq_ps")
                        for kc in range(KC):
                            sq = p3.tile([128, HR], F32R, tag="sq")
                            nc.vector.tensor_mul(sq, ctxF[:, kc, h, :],
                                                 ctxF[:, kc, h, :])
                            nc.tensor.matmul(
                                sq_ps[:, :], ones_col_r, sq,
                                start=(kc == 0), stop=(kc == KC - 1))
                        # reshape small stats to all 128 DVE lanes
                        ssb = p3s.tile([1, 2, HR], F32, tag="ssb")
                        nc.scalar.copy(ssb[:, 0, :], sum_ps[:, :])
                        nc.scalar.copy(ssb[:, 1, :], sq_ps[:, :])
                        s_sum = p3s.tile([128, 2], F32, tag="s_sum")
                        nc.scalar.dma_start(out=s_sum[:, :], in_=ssb[:, 0, :])
                        s_sq = p3s.tile([128, 2], F32, tag="s_sq")
                        nc.scalar.dma_start(out=s_sq[:, :], in_=ssb[:, 1, :])
                        mean8 = p3s.tile([128, 2], F32, tag="mean8")
                        nc.vector.tensor_scalar_mul(mean8, s_sum[:, :],
                                                    1.0 / D)
                        m28 = p3s.tile([128, 2], F32, tag="m28")
                        nc.vector.tensor_mul(m28, mean8, mean8)
                        u8 = p3s.tile([128, 2], F32, tag="u8")
                        nc.vector.tensor_scalar(
                            out=u8, in0=s_sq[:, :], scalar1=1.0 / D,
                            scalar2=EPS, op0=OP.mult, op1=OP.add)
                        nc.vector.tensor_sub(u8, u8, m28)
                        r8 = _rsqrt_dve(nc, p3s, u8, 128, 2, "oln")
                        negm8 = p3s.tile([128, 2], F32R, tag="negm8")
                        with nc.allow_low_precision(
                                reason="fp32r is 32-bit storage"):
                            nc.vector.tensor_mul(negm8, mean8,
                                                 r8.bitcast(F32))
                            nc.vector.tensor_scalar_mul(negm8, negm8, -1.0)
                        rstd = p3s.tile([1, HR], F32R, tag="rstd")
                        nc.scalar.dma_start(out=rstd[:, :], in_=r8[:, :])
                        negm = p3s.tile([1, HR], F32R, tag="negm")
                        nc.scalar.dma_start(out=negm[:, :], in_=negm8[:, :])
                        rstd_ps = st_ps.tile([128, HR], F32, tag="rstd_ps",
                                             name="rstd_ps")
                        nc.tensor.matmul(
                            rstd_ps[:, :], ones_row, rstd,
                            start=True, stop=True)
                        negm_ps = st_ps.tile([128, HR], F32, tag="negm_ps",
                                             name="negm_ps")
                        nc.tensor.matmul(
                            negm_ps[:, :], ones_row, negm,
                            start=True, stop=True)

                        ctxn = p3w.tile([128, KC, HR], F32R, tag="ctxn")
                        for kc in range(KC):
                            t = p3.tile([128, HR], F32, tag="lnt")
                            nc.vector.tensor_mul(t, ctxF[:, kc, h, :],
                                                 rstd_ps[:, :])
                            with nc.allow_low_precision(
                                    reason="fp32r is 32-bit storage"):
                                nc.vector.tensor_add(ctxn[:, kc, :], t,
                                                     negm_ps[:, :])

                        for m in range(HR // 128):
                            ps = o_ps.tile([128, D], F32, tag="o")
                            for n in range(2):
                                for kc in range(KC):
                                    nc.tensor.matmul(
                                        ps[:, n * 512:(n + 1) * 512],
                                        ctxn[:, kc, m * 128:(m + 1) * 128],
                                        wp_t[:, kc, n * 512:(n + 1) * 512],
                                        start=(kc == 0), stop=(kc == KC - 1))
                            o_sb = p3.tile([128, D], F32, tag="osb")
                            nc.vector.tensor_add(o_sb, ps[:, :], bvb)
                            nc.sync.dma_start(
                                out=out[h * HR + m * 128:
                                        h * HR + (m + 1) * 128, :],
                                in_=o_sb)
    nc.finalize()
    return nc


def _center_per_head(W):
    Wc = np.asarray(W, np.float32).reshape(D, H, HD)
    return (Wc - Wc.mean(axis=2, keepdims=True)).reshape(D, D)


def make_in_maps(x_q, x_k, x_v, Wq, Wk, Wv, Wproj,
                 q_gamma, q_beta, k_gamma, k_beta, out_gamma, out_beta):
    """Host-side prep: transpose inputs, center + slice weights, fold scale,
    fold out-LN gamma/beta into Wproj / a bias vector."""
    bf = ml_dtypes.bfloat16
    scale = np.float32(HD ** -0.5)
    xqT = np.ascontiguousarray(
        np.asarray(x_q, np.float32).transpose(0, 2, 1)).astype(bf)
    xkT = np.ascontiguousarray(
        np.asarray(x_k, np.float32).transpose(0, 2, 1)).astype(bf)
    xvT = np.ascontiguousarray(
        np.asarray(x_v, np.float32).transpose(0, 2, 1)).astype(bf)
    WqC = _center_per_head(Wq)
    WkC = _center_per_head(Wk)
    Wv = np.ascontiguousarray(np.asarray(Wv, np.float32))
    Wp = np.asarray(Wproj, np.float32)
    og = np.asarray(out_gamma, np.float32)
    ob = np.asarray(out_beta, np.float32)
    Wp2 = np.ascontiguousarray(Wp * og[:, None])
    bvec = ob @ Wp
    qg2 = np.tile(np.asarray(q_gamma, np.float32) * scale, HPC)
    qb2 = np.tile(np.asarray(q_beta, np.float32) * scale, HPC)
    kg2 = np.tile(np.asarray(k_gamma, np.float32), HPC)
    kb2 = np.tile(np.asarray(k_beta, np.float32), HPC)
    bo2 = np.zeros((128, 2), np.float32)
    bo2[0:64, 0] = 1.0
    bo2[64:128, 1] = 1.0
    cb2 = np.zeros((2, 128), np.float32)
    cb2[0, 0:64] = 1.0
    cb2[1, 64:128] = 1.0
    idp = np.eye(128, dtype=np.float32).astype(bf)
    in_maps = []
    for c in range(NCORES):
        dc = slice(DPC * c, DPC * (c + 1))
        in_maps.append({
            "xqT": xqT, "xkT": xkT, "xvT": xvT,
            "wq": np.ascontiguousarray(WqC[:, dc]).astype(bf),
            "wk": np.ascontiguousarray(WkC[:, dc]).astype(bf),
            "wv": np.ascontiguousarray(Wv[:, dc]).astype(bf),
            "wp": Wp2, "bv": bvec,
            "qg": qg2, "qb": qb2, "kg": kg2, "kb": kb2,
            "bo2": bo2, "cb2": cb2, "idp": idp,
        })
    return in_maps


_NC_CACHE = None


def _get_nc():
    global _NC_CACHE
    if _NC_CACHE is None:
        _NC_CACHE = build(1)
    return _NC_CACHE


def kernel(x_q, x_k, x_v, Wq, Wk, Wv, Wproj,
           q_gamma, q_beta, k_gamma, k_beta, out_gamma, out_beta,
           _trace=False):
    in_maps = make_in_maps(x_q, x_k, x_v, Wq, Wk, Wv, Wproj,
                           q_gamma, q_beta, k_gamma, k_beta,
                           out_gamma, out_beta)
    nc = _get_nc()
    res = bass_utils.run_bass_kernel_spmd(
        nc, in_maps, list(range(NCORES)), trace=_trace)
    full = np.empty((B, S, D), dtype=np.float32)
    for c in range(NCORES):
        b, r = c // 4, c % 4
        full[b, r * OUT_ROWS:(r + 1) * OUT_ROWS, :] = res.results[c]["out"]
    if _trace:
        return full, res
    return full
